# revision 39
# baseline (speedup 1.0000x reference)
"""Trainium2 Bass kernel for CrossAttention.

Problem shape (hardcoded):
  latent  [8, 4096, 512], context [8, 77, 768]
  wq [512,512], wk/wv [768,512], wo [512,512], biases [512]
  out = softmax((latent@wq+bq)(context@wk+bk)^T / 8) @ (context@wv+bv) @ wo + bo

Sharding: data-parallel over batch - core b handles batch element b.

Structure (all-bf16 matmuls; fp8 rejected: e4m3 adds ~3-5% rel err vs the
2e-2 budget):
  * x transposed via XBAR DMA on the sync queue, issued TWO iterations
    ahead of use (the [128,512] xbar transpose decomposes into 512x256B
    descriptors and needs the slack) - no PE transposes, no ACT copies.
  * PSUM banks (8): qT ring 2x1, out ring 2x1, scores+sums ring 2x1,
    attnT per-HALF ring 2x1.  One pool per ring - the scheduler's
    per-engine counters otherwise entangle unrelated stages.
  * per-256-row iteration PE columns: qproj 16x256 + scores 8x256 +
    sums 8x256 + PV 8x256 + outproj 8x512 + bias 2x512 = 15360
    (~6.4us at 0.417 ns/col); measured span ~8.4us/iter - the gap is
    cross-engine latency (exp -> sums, divide -> outproj) plus the
    tile-clock coupling of successive iterations.
  * softmax normalize: recip_approx_fast + tensor_mul per half (DVE),
    per-half attnT SBUF tiles keep outproj deps tile-precise.
  * out bias bo via rank-1 e0 matmul into PSUM (PE), out copy on ACT.
    (gp tensor_add can't read PSUM; DVE add and gp add both measured
    slower end-to-end - they shift the inter-iteration barrier onto a
    busier engine.)
  * startup: make_identity first (it runs on the gpsimd queue and gates
    the first PE work); ctx DMA carries zero waits (padding zeroed in cT
    columns instead of ctx rows) so the scheduler cannot hoist weight
    loads ahead of it; weights before x tiles - compute starting
    input-starved loses more to barrier cascades than the head start
    gains (first matmul at ~11.4us).

Engine budget per iter: PE 6.4us, DVE qbias 4x483 + recip/mul 4x675 =
4.6us, ACT exp 4x705 + outcopy 2x705 = 4.2us, Sync xpose 2x1206 +
store 2x641 = 3.7us, GpSimd xload 0.8us.

Measured: ~157-161us median, best runs 154-157us; HW exec is bimodal
~+/-4% run to run on the same NEFF (baseline: 164-166us).
"""

import os
import sys
from contextlib import ExitStack

import numpy as np

for _p in ("/opt/trn_rl_repo",):
    if _p not in sys.path and os.path.isdir(_p):
        sys.path.insert(0, _p)

import concourse.bass as bass  # noqa: E402
import concourse.tile as tile  # noqa: E402
from concourse import bacc, mybir  # noqa: E402
from concourse.bass_utils import run_bass_kernel_spmd  # noqa: E402
from concourse.masks import make_identity  # noqa: E402

N_CORES = 8
SQ, D, DC, SKV, H, DH = 4096, 512, 768, 77, 8, 64
F32 = mybir.dt.float32
BF16 = mybir.dt.bfloat16
AF = mybir.ActivationFunctionType
ALU = mybir.AluOpType

# ---- config flags (A/B-able via env) ----
XT_MODE = os.environ.get("CA_XT", "xbar")   # xbar | pe
OB_MODE = os.environ.get("CA_OB", "pe")     # pe | gp | dve  (out-bias engine)
NROW = 256                       # rows per iteration
NCH = NROW // 128                # 128-row chunks per iteration


def build_nc(n_iters=SQ // NROW):
    nc = bacc.Bacc("TRN2", target_bir_lowering=False, debug=False)

    lat = nc.dram_tensor("latent", [SQ, D], F32, kind="ExternalInput").ap()
    ctx_d = nc.dram_tensor("context", [SKV, DC], F32, kind="ExternalInput").ap()
    wq = nc.dram_tensor("wq", [D, D], F32, kind="ExternalInput").ap()
    bq = nc.dram_tensor("bq", [D], F32, kind="ExternalInput").ap()
    wk = nc.dram_tensor("wk", [DC, D], F32, kind="ExternalInput").ap()
    bk = nc.dram_tensor("bk", [D], F32, kind="ExternalInput").ap()
    wv = nc.dram_tensor("wv", [DC, D], F32, kind="ExternalInput").ap()
    bv = nc.dram_tensor("bv", [D], F32, kind="ExternalInput").ap()
    wo = nc.dram_tensor("wo", [D, D], F32, kind="ExternalInput").ap()
    bo = nc.dram_tensor("bo", [D], F32, kind="ExternalInput").ap()
    out_d = nc.dram_tensor("out", [SQ, D], F32, kind="ExternalOutput").ap()

    with tile.TileContext(nc) as tc:
        with ExitStack() as stk:
            consts = stk.enter_context(tc.tile_pool(name="consts", bufs=1))
            prep = stk.enter_context(tc.tile_pool(name="prep", bufs=1))
            xpool = stk.enter_context(tc.tile_pool(name="x", bufs=3))
            spool = stk.enter_context(tc.tile_pool(name="work", bufs=2))
            opool = stk.enter_context(tc.tile_pool(name="outp", bufs=3))
            # PSUM (8 banks): psq "q" ring (2x1), pso "out" ring (2x1),
            # pss "sT" scores+sums ring (2x1), psw "pv" ring (2x1).
            # Separate pools per ring: a pool's ring clock is shared
            # across its tags, which would serialize unrelated stages.
            psq = stk.enter_context(tc.tile_pool(name="psq", bufs=2, space="PSUM"))
            pso = stk.enter_context(tc.tile_pool(name="pso", bufs=2, space="PSUM"))
            pss = stk.enter_context(tc.tile_pool(name="pss", bufs=2, space="PSUM"))
            psw = stk.enter_context(tc.tile_pool(name="psw", bufs=2, space="PSUM"))

            # ---------------- constants ----------------
            # x loads + weights on the SWDGE queue, ordered by first use:
            # ctx (cT prep), x0 (front 0), wk (kT prep), wq (qproj 0),
            # x1, wv (PV 0), x2, wo (outproj 0, needed one body later).
            x_tiles = {}
            n_loop_iters = n_iters

            def load_x(it):
                if it >= n_loop_iters:
                    return
                x_sb = xpool.tile([128, NCH, D], BF16, tag="x", bufs=4,
                                  name="x_sb")
                nc.gpsimd.dma_start(
                    x_sb, lat[it * NROW : (it + 1) * NROW, :].rearrange(
                        "(c p) d -> p c d", p=128
                    )
                )
                x_tiles[it] = x_sb

            # identity FIRST: make_identity uses the gpsimd queue, and the
            # cT transposes (first PE work) need it - it must not queue
            # behind the heavy weight/x DMA issues below.
            ident = consts.tile([128, 128], BF16, name="ident")
            make_identity(nc, ident)
            # small constant tiles (vector memsets, cheap, early)
            # ctx rows >= 77 stay garbage: the ctx DMA must carry NO waits
            # (any dependency makes the scheduler hoist the heavy weight
            # loads ahead of it on the SWDGE queue).  The garbage rows
            # transpose into cT columns >= 77, which are zeroed there.
            ctx_sb = prep.tile([128, DC], BF16, name="ctx_sb")
            e0 = consts.tile([128, 128], BF16, name="e0")
            nc.vector.memset(e0, 0.0)
            nc.vector.memset(e0[0:1, :], 1.0)
            ones64 = consts.tile([128, DH], BF16, name="ones64")
            nc.vector.memset(ones64, 0.0)
            nc.vector.memset(ones64[:SKV, :], 1.0)
            bv_pad = consts.tile([128, D], BF16, name="bv_pad")
            nc.vector.memset(bv_pad, 0.0)
            bo_pad = consts.tile([128, D], BF16, name="bo_pad")
            nc.vector.memset(bo_pad, 0.0)
            # bq pair-major: partition p, col g  <-> bq[g*128 + p]
            bq_pair = consts.tile([128, 4], F32, name="bq_pair")
            nc.sync.dma_start(bq_pair, bq.rearrange("(g p) -> p g", p=128))
            # bk head-major halves, prescaled by 1/8: partition q*64+d, col g
            # <-> bk[(2g+q)*64 + d] * 0.125
            bk_hm = consts.tile([128, 4], F32, name="bk_hm")
            nc.sync.dma_start(bk_hm, bk.rearrange("(g q d) -> (q d) g", g=4, q=2))
            bk_hms = consts.tile([128, 4], F32, name="bk_hms")
            nc.vector.tensor_scalar_mul(bk_hms, bk_hm, 0.125)

            # heavy SWDGE loads: weights first (compute starting input-
            # starved costs more in barrier cascades than it saves), x after.
            # ctx is tiny and releases the whole cT/kT prep chain - force it
            # to the actual front of the DMA stream.
            with tc.high_priority():
                nc.gpsimd.dma_start(ctx_sb[:SKV, :], ctx_d)
            wk_sb = consts.tile([128, 6, D], BF16, name="wk_sb")
            nc.gpsimd.dma_start(wk_sb, wk.rearrange("(t p) d -> p t d", p=128))
            wq_sb = consts.tile([128, 4, D], BF16, name="wq_sb")
            nc.gpsimd.dma_start(wq_sb, wq.rearrange("(t p) d -> p t d", p=128))
            wv_sb = consts.tile([128, 6, D], BF16, name="wv_sb")
            nc.gpsimd.dma_start(wv_sb, wv.rearrange("(t p) d -> p t d", p=128))
            wo_sb = consts.tile([128, 4, D], BF16, name="wo_sb")
            nc.gpsimd.dma_start(wo_sb, wo.rearrange("(t p) d -> p t d", p=128))
            nc.gpsimd.dma_start(bv_pad[0:1, :], bv.rearrange("(o d) -> o d", o=1))
            nc.gpsimd.dma_start(bo_pad[0:1, :], bo.rearrange("(o d) -> o d", o=1))
            load_x(0)
            load_x(1)
            load_x(2)
            if OB_MODE in ("gp", "dve"):
                # bo broadcast across partitions via K=1 matmul:
                # e0[0:1,:] (ones row) x bo_pad[0:1,:] -> [128, D]
                bo_ps = pso.tile([128, D], F32, tag="out", name="bo_ps")
                nc.tensor.matmul(
                    bo_ps, lhsT=e0[0:1, :], rhs=bo_pad[0:1, :],
                    start=True, stop=True,
                )
                bo_bcast = consts.tile([128, D], F32, name="bo_bcast")
                nc.vector.tensor_copy(bo_bcast, bo_ps)

            # ---------------- K/V prep (once) ----------------
            # cT [128, 6, 128]; cols >= 77 zeroed directly (free-dim slice)
            cT_sb = prep.tile([128, 6, 128], BF16, name="cT_sb")
            nc.vector.memset(cT_sb[:, :, SKV:], 0.0)
            for g in range(2):
                cT_ps = pss.tile([128, 3, 128], BF16, tag="sT", name="cT_ps")
                for t3 in range(3):
                    t = g * 3 + t3
                    nc.tensor.transpose(
                        cT_ps[:, t3, :], ctx_sb[:, t * 128 : (t + 1) * 128], ident
                    )
                nc.vector.tensor_copy(
                    cT_sb[:, 3 * g : 3 * g + 3, :SKV], cT_ps[:, :, :SKV]
                )

            # kT per head, masked into pair halves:
            #   head h lives in partitions (h%2)*64..+64 of kT_sb[:, h, :];
            #   the other 64 partitions are zero; cols 77..127 zero.
            kT_sb = prep.tile([128, H, 128], BF16, name="kT_sb")
            nc.vector.memset(kT_sb, 0.0)
            for g in range(4):
                kT_ps = psq.tile([128, 128], F32, tag="q", name="kT_ps")
                for q in range(2):
                    h = 2 * g + q
                    off = q * 64
                    for ct in range(6):
                        nc.tensor.matmul(
                            kT_ps[off : off + 64, :],
                            lhsT=wk_sb[:, ct, h * 64 : (h + 1) * 64],
                            rhs=cT_sb[:, ct, :],
                            start=(ct == 0),
                            stop=(ct == 5),
                        )
                for q in range(2):
                    h = 2 * g + q
                    off = q * 64
                    nc.scalar.activation(
                        kT_sb[off : off + 64, h, :SKV],
                        kT_ps[off : off + 64, :SKV],
                        AF.Identity,
                        bias=bk_hms[off : off + 64, g : g + 1],
                        scale=0.125,
                    )

            # v [kv, d] with rows >= 77 exactly 0 (zero cT cols + e0[:, :77])
            # built lazily in body 0 (after front(0)) so qproj(0) does not
            # queue behind the wv DMA on the PE.
            state = {}

            def prep_v():
                v_ps = pso.tile([128, D], F32, tag="out", name="v_ps")
                for ct in range(6):
                    nc.tensor.matmul(
                        v_ps,
                        lhsT=cT_sb[:, ct, :],
                        rhs=wv_sb[:, ct, :],
                        start=(ct == 0),
                        stop=False,
                    )
                nc.tensor.matmul(
                    v_ps, lhsT=e0, rhs=bv_pad, start=False, stop=True
                )
                # rows >= 77 got bv from the rank-1 bias matmul; zero them
                # (they must mask the exp(0)=1 padding rows in PV).
                v_sb = prep.tile([128, D], BF16, name="v_sb")
                nc.vector.memset(v_sb[64:128, :], 0.0)
                nc.vector.tensor_copy(v_sb[:SKV, :], v_ps[:SKV, :])
                state["v_sb"] = v_sb

            # ---------------- main loop (software-pipelined) ----------
            # Emission order per body it:
            #   load x(it+2) | xpose(it+1) | qproj+scores+exp(it)
            #   | outproj(it-1)+store | sums/PV/normalize(it)
            # PE in-order queue: Q(it) S(it) O(it-1) Sums/PV(it) - each
            # cross-engine latency hop (qbias DVE, exp ACT, divide DVE)
            # is covered by the next block of independent PE work.
            xT_tiles = {}

            def xpose(it):
                if it >= n_iters:
                    return
                x_sb = x_tiles.pop(it)
                xT_sb = xpool.tile([128, 4, NROW], BF16, tag="xT", bufs=3,
                                   name="xT_sb")
                if XT_MODE == "xbar":
                    for c in range(NCH):
                        nc.sync.dma_start_transpose(
                            xT_sb[:, :, c * 128 : (c + 1) * 128], x_sb[:, c, :]
                        )
                else:
                    for c in range(NCH):
                        xT_ps = psq.tile([128, 4, 128], BF16, tag="q",
                                         name="xT_ps")
                        for et in range(4):
                            nc.tensor.transpose(
                                xT_ps[:, et, :],
                                x_sb[:, c, et * 128 : (et + 1) * 128],
                                ident,
                            )
                        nc.scalar.copy(xT_sb[:, :, c * 128 : (c + 1) * 128], xT_ps)
                xT_tiles[it] = xT_sb

            def stage_front(it):
                """qproj + scores + exp for iter it."""
                xT_sb = xT_tiles.pop(it)
                qT_sb = spool.tile([128, 4, NROW], BF16, tag="qT", name="qT_sb")
                for g in range(4):
                    qT_ps = psq.tile([128, NROW], F32, tag="q", name="qT_ps")
                    for et in range(4):
                        nc.tensor.matmul(
                            qT_ps,
                            lhsT=wq_sb[:, et, g * 128 : (g + 1) * 128],
                            rhs=xT_sb[:, et, :],
                            start=(et == 0),
                            stop=(et == 3),
                        )
                    # single-ALU-op add: a mult-by-1.0 + add costs ~100ns
                    # more per op and delays the DVE queue tail
                    nc.vector.tensor_scalar_add(
                        qT_sb[:, g, :], qT_ps, bq_pair[:, g : g + 1]
                    )

                expT_sb = spool.tile([128, H, NROW], BF16, tag="expT", name="expT_sb")
                for g in range(4):
                    sT_ps = pss.tile([128, 2, NROW], F32, tag="sT", name="sT_ps")
                    for q in range(2):
                        nc.tensor.matmul(
                            sT_ps[:, q, :],
                            lhsT=kT_sb[:, 2 * g + q, :],
                            rhs=qT_sb[:, g, :],
                            start=True,
                            stop=True,
                        )
                    nc.scalar.activation(
                        expT_sb[:, 2 * g : 2 * g + 2, :], sT_ps, AF.Exp
                    )
                return expT_sb

            def stage_back(it, expT_sb):
                """sums + PV + softmax normalize for iter it.

                Returns per-half attnT SBUF tiles [128, 2, NROW] (bf16):
                half h covers heads 4h..4h+3 packed (h%2)*64 partitions,
                free index dt-within-half.
                """
                halves = []
                for half in range(2):
                    sums_ps = pss.tile([128, 2, NROW], F32, tag="sT", name="sums_ps")
                    pv_ps = psw.tile([128, 2, NROW], F32, tag="pv", name="pv_ps")
                    for hh in range(4):
                        h = half * 4 + hh
                        dt, off = hh // 2, (h % 2) * 64
                        nc.tensor.matmul(
                            sums_ps[off : off + 64, dt, :],
                            lhsT=ones64,
                            rhs=expT_sb[:, h, :],
                            start=True,
                            stop=True,
                        )
                    for hh in range(4):
                        h = half * 4 + hh
                        dt, off = hh // 2, (h % 2) * 64
                        nc.tensor.matmul(
                            pv_ps[off : off + 64, dt, :],
                            lhsT=state["v_sb"][:, h * 64 : (h + 1) * 64],
                            rhs=expT_sb[:, h, :],
                            start=True,
                            stop=True,
                        )
                    rsum_sb = spool.tile([128, 2, NROW], F32, tag="rsum",
                                         name="rsum_sb")
                    nc.vector.reciprocal_approx_fast(rsum_sb, sums_ps)
                    a_sb = spool.tile([128, 2, NROW], BF16, tag="attnT", bufs=4,
                                      name="a_sb")
                    nc.vector.tensor_mul(a_sb, pv_ps, rsum_sb)
                    halves.append(a_sb)
                return halves

            def stage_out(it, halves):
                """out projection + bias + store for iter it."""
                for c in range(NCH):
                    out_ps = pso.tile([128, D], F32, tag="out", name="out_ps")
                    for half in range(2):
                        for j in range(2):
                            dt = 2 * half + j
                            nc.tensor.matmul(
                                out_ps,
                                lhsT=halves[half][:, j, c * 128 : (c + 1) * 128],
                                rhs=wo_sb[:, dt, :],
                                start=(dt == 0),
                                stop=(OB_MODE != "pe" and dt == 3),
                            )
                    out_sb = opool.tile([128, D], F32, tag="out", name="out_sb")
                    if OB_MODE == "dve":
                        # one DVE op: PSUM->SBUF copy + bo add. No ACT copy,
                        # no PE bias matmul - the scalar queue is exps-only.
                        nc.vector.tensor_tensor(out_sb, out_ps, bo_bcast, ALU.add)
                        st = out_sb
                    elif OB_MODE == "gp":
                        nc.scalar.copy(out_sb, out_ps)
                        out2_sb = opool.tile([128, D], F32, tag="out2",
                                             name="out2_sb")
                        nc.gpsimd.tensor_add(out2_sb, out_sb, bo_bcast)
                        st = out2_sb
                    else:
                        nc.tensor.matmul(
                            out_ps, lhsT=e0, rhs=bo_pad, start=False, stop=True
                        )
                        # DVE copy (418ns) vs ACT copy (705ns): also empties
                        # the scalar queue down to exps only, removing the
                        # scalar-side inter-iteration barrier
                        nc.vector.tensor_copy(out_sb, out_ps)
                        st = out_sb
                    nc.sync.dma_start(
                        out_d[it * NROW + c * 128 : it * NROW + (c + 1) * 128, :],
                        st,
                    )

            xpose(0)
            xpose(1)
            prev = None
            for it in range(n_iters):
                load_x(it + 3)
                xpose(it + 2)
                expT = stage_front(it)
                if it == 0:
                    prep_v()
                if prev is not None:
                    stage_out(it - 1, prev)
                prev = stage_back(it, expT)
            stage_out(n_iters - 1, prev)

    nc.compile()
    return nc


_BUILD_CACHE = {}


def _get_nc():
    key = (XT_MODE, OB_MODE)
    if key not in _BUILD_CACHE:
        _BUILD_CACHE[key] = build_nc()
    return _BUILD_CACHE[key]


def _in_maps(latent, context, wq, bq, wk, bk, wv, bv, wo, bo):
    f = lambda a: np.ascontiguousarray(np.asarray(a), dtype=np.float32)
    shared = {
        "wq": f(wq), "bq": f(bq), "wk": f(wk), "bk": f(bk),
        "wv": f(wv), "bv": f(bv), "wo": f(wo), "bo": f(bo),
    }
    maps = []
    for b in range(N_CORES):
        m = dict(shared)
        m["latent"] = f(latent[b])
        m["context"] = f(context[b])
        maps.append(m)
    return maps


def run_on_hw(inputs, trace=False, **kw):
    nc = _get_nc()
    maps = _in_maps(**inputs)
    res = run_bass_kernel_spmd(nc, maps, list(range(N_CORES)), trace=trace, **kw)
    out = np.stack([res.results[b]["out"] for b in range(N_CORES)], axis=0)
    return out, res


def kernel(latent, context, wq, bq, wk, bk, wv, bv, wo, bo):
    out, _ = run_on_hw(dict(
        latent=latent, context=context, wq=wq, bq=bq, wk=wk, bk=bk,
        wv=wv, bv=bv, wo=wo, bo=bo,
    ))
    return out


# revision 40
# speedup vs baseline: 1.2634x; 1.2634x over previous
"""Trainium2 Bass kernel for CrossAttention.

Problem shape (hardcoded):
  latent  [8, 4096, 512], context [8, 77, 768]
  wq [512,512], wk/wv [768,512], wo [512,512], biases [512]
  out = softmax((latent@wq+bq)(context@wk+bk)^T / 8) @ (context@wv+bv) @ wo + bo

Sharding: data-parallel over batch - core b handles batch element b.

Structure (all-bf16 matmuls; fp8 rejected: e4m3 adds ~3-5% rel err vs the
2e-2 budget):
  * x transposed via XBAR DMA on the sync queue, issued TWO iterations
    ahead of use (the [128,512] xbar transpose decomposes into 512x256B
    descriptors and needs the slack) - no PE transposes, no ACT copies.
  * PSUM banks (8): qT ring 2x1, out ring 2x1, scores+sums ring 2x1,
    attnT per-HALF ring 2x1.  One pool per ring - the scheduler's
    per-engine counters otherwise entangle unrelated stages.
  * per-256-row iteration PE columns: qproj 16x256 + scores 8x256 +
    sums 8x256 + PV 8x256 + outproj 8x512 + bias 2x512 = 15360
    (~6.4us at 0.417 ns/col); measured span ~8.4us/iter - the gap is
    cross-engine latency (exp -> sums, divide -> outproj) plus the
    tile-clock coupling of successive iterations.
  * softmax normalize: recip_approx_fast + tensor_mul per half (DVE),
    per-half attnT SBUF tiles keep outproj deps tile-precise.
  * out bias bo via rank-1 e0 matmul into PSUM (PE), out copy on ACT.
    (gp tensor_add can't read PSUM; DVE add and gp add both measured
    slower end-to-end - they shift the inter-iteration barrier onto a
    busier engine.)
  * startup: make_identity first (it runs on the gpsimd queue and gates
    the first PE work); ctx DMA carries zero waits (padding zeroed in cT
    columns instead of ctx rows) so the scheduler cannot hoist weight
    loads ahead of it; weights before x tiles - compute starting
    input-starved loses more to barrier cascades than the head start
    gains (first matmul at ~11.4us).

Engine budget per iter: PE 6.4us, DVE qbias 4x483 + recip/mul 4x675 =
4.6us, ACT exp 4x705 + outcopy 2x705 = 4.2us, Sync xpose 2x1206 +
store 2x641 = 3.7us, GpSimd xload 0.8us.

Measured: ~157-161us median, best runs 154-157us; HW exec is bimodal
~+/-4% run to run on the same NEFF (baseline: 164-166us).
"""

import os
import sys
from contextlib import ExitStack

import numpy as np

for _p in ("/opt/trn_rl_repo",):
    if _p not in sys.path and os.path.isdir(_p):
        sys.path.insert(0, _p)

import concourse.bass as bass  # noqa: E402
import concourse.tile as tile  # noqa: E402
from concourse import bacc, mybir  # noqa: E402
from concourse.bass_utils import run_bass_kernel_spmd  # noqa: E402
from concourse.masks import make_identity  # noqa: E402

N_CORES = 8
SQ, D, DC, SKV, H, DH = 4096, 512, 768, 77, 8, 64
F32 = mybir.dt.float32
BF16 = mybir.dt.bfloat16
AF = mybir.ActivationFunctionType
ALU = mybir.AluOpType

# ---- config flags (A/B-able via env) ----
XT_MODE = os.environ.get("CA_XT", "xbar")   # xbar | pe
OB_MODE = os.environ.get("CA_OB", "pe")     # pe | gp | dve  (out-bias engine)
NROW = 256                       # rows per iteration
NCH = NROW // 128                # 128-row chunks per iteration


def build_nc(n_iters=SQ // NROW):
    nc = bacc.Bacc("TRN2", target_bir_lowering=False, debug=False)

    lat = nc.dram_tensor("latent", [SQ, D], F32, kind="ExternalInput").ap()
    ctx_d = nc.dram_tensor("context", [SKV, DC], F32, kind="ExternalInput").ap()
    wq = nc.dram_tensor("wq", [D, D], F32, kind="ExternalInput").ap()
    bq = nc.dram_tensor("bq", [D], F32, kind="ExternalInput").ap()
    wk = nc.dram_tensor("wk", [DC, D], F32, kind="ExternalInput").ap()
    bk = nc.dram_tensor("bk", [D], F32, kind="ExternalInput").ap()
    wv = nc.dram_tensor("wv", [DC, D], F32, kind="ExternalInput").ap()
    bv = nc.dram_tensor("bv", [D], F32, kind="ExternalInput").ap()
    wo = nc.dram_tensor("wo", [D, D], F32, kind="ExternalInput").ap()
    bo = nc.dram_tensor("bo", [D], F32, kind="ExternalInput").ap()
    out_d = nc.dram_tensor("out", [SQ, D], F32, kind="ExternalOutput").ap()

    with tile.TileContext(nc) as tc:
        with ExitStack() as stk:
            consts = stk.enter_context(tc.tile_pool(name="consts", bufs=1))
            prep = stk.enter_context(tc.tile_pool(name="prep", bufs=1))
            xpool = stk.enter_context(tc.tile_pool(name="x", bufs=3))
            spool = stk.enter_context(tc.tile_pool(name="work", bufs=2))
            opool = stk.enter_context(tc.tile_pool(name="outp", bufs=3))
            # PSUM (8 banks): psq "q" ring (2x1), pso "out" ring (2x1),
            # pss "sT" scores+sums ring (2x1), psw "pv" ring (2x1).
            # Separate pools per ring: a pool's ring clock is shared
            # across its tags, which would serialize unrelated stages.
            psq = stk.enter_context(tc.tile_pool(name="psq", bufs=2, space="PSUM"))
            pso = stk.enter_context(tc.tile_pool(name="pso", bufs=2, space="PSUM"))
            pss = stk.enter_context(tc.tile_pool(name="pss", bufs=2, space="PSUM"))
            psw = stk.enter_context(tc.tile_pool(name="psw", bufs=2, space="PSUM"))

            # ---------------- constants ----------------
            # x loads + weights on the SWDGE queue, ordered by first use:
            # ctx (cT prep), x0 (front 0), wk (kT prep), wq (qproj 0),
            # x1, wv (PV 0), x2, wo (outproj 0, needed one body later).
            x_tiles = {}
            n_loop_iters = n_iters

            def load_x(it):
                if it >= n_loop_iters:
                    return
                x_sb = xpool.tile([128, NCH, D], BF16, tag="x", bufs=4,
                                  name="x_sb")
                nc.gpsimd.dma_start(
                    x_sb, lat[it * NROW : (it + 1) * NROW, :].rearrange(
                        "(c p) d -> p c d", p=128
                    )
                )
                x_tiles[it] = x_sb

            # identity FIRST: make_identity uses the gpsimd queue, and the
            # cT transposes (first PE work) need it - it must not queue
            # behind the heavy weight/x DMA issues below.
            ident = consts.tile([128, 128], BF16, name="ident")
            make_identity(nc, ident)
            # small constant tiles (vector memsets, cheap, early)
            # ctx rows >= 77 stay garbage: the ctx DMA must carry NO waits
            # (any dependency makes the scheduler hoist the heavy weight
            # loads ahead of it on the SWDGE queue).  The garbage rows
            # transpose into cT columns >= 77, which are zeroed there.
            ctx_sb = prep.tile([128, DC], BF16, name="ctx_sb")
            e0 = consts.tile([128, 128], BF16, name="e0")
            nc.vector.memset(e0, 0.0)
            nc.vector.memset(e0[0:1, :], 1.0)
            ones64 = consts.tile([128, DH], BF16, name="ones64")
            nc.vector.memset(ones64, 0.0)
            nc.vector.memset(ones64[:SKV, :], 1.0)
            bv_pad = consts.tile([128, D], BF16, name="bv_pad")
            nc.vector.memset(bv_pad, 0.0)
            bo_pad = consts.tile([128, D], BF16, name="bo_pad")
            nc.vector.memset(bo_pad, 0.0)
            # bq pair-major: partition p, col g  <-> bq[g*128 + p]
            bq_pair = consts.tile([128, 4], F32, name="bq_pair")
            nc.sync.dma_start(bq_pair, bq.rearrange("(g p) -> p g", p=128))
            # bk head-major halves, prescaled by 1/8: partition q*64+d, col g
            # <-> bk[(2g+q)*64 + d] * 0.125
            bk_hm = consts.tile([128, 4], F32, name="bk_hm")
            nc.sync.dma_start(bk_hm, bk.rearrange("(g q d) -> (q d) g", g=4, q=2))
            bk_hms = consts.tile([128, 4], F32, name="bk_hms")
            nc.vector.tensor_scalar_mul(bk_hms, bk_hm, 0.125)

            # heavy SWDGE loads: weights first (compute starting input-
            # starved costs more in barrier cascades than it saves), x after.
            # ctx is tiny and releases the whole cT/kT prep chain - force it
            # to the actual front of the DMA stream.
            with tc.high_priority():
                nc.gpsimd.dma_start(ctx_sb[:SKV, :], ctx_d)
            wk_sb = consts.tile([128, 6, D], BF16, name="wk_sb")
            nc.gpsimd.dma_start(wk_sb, wk.rearrange("(t p) d -> p t d", p=128))
            wq_sb = consts.tile([128, 4, D], BF16, name="wq_sb")
            nc.gpsimd.dma_start(wq_sb, wq.rearrange("(t p) d -> p t d", p=128))
            wv_sb = consts.tile([128, 6, D], BF16, name="wv_sb")
            nc.gpsimd.dma_start(wv_sb, wv.rearrange("(t p) d -> p t d", p=128))
            wo_sb = consts.tile([128, 4, D], BF16, name="wo_sb")
            nc.gpsimd.dma_start(wo_sb, wo.rearrange("(t p) d -> p t d", p=128))
            nc.gpsimd.dma_start(bv_pad[0:1, :], bv.rearrange("(o d) -> o d", o=1))
            nc.gpsimd.dma_start(bo_pad[0:1, :], bo.rearrange("(o d) -> o d", o=1))
            load_x(0)
            load_x(1)
            load_x(2)
            if OB_MODE in ("gp", "dve"):
                # bo broadcast across partitions via K=1 matmul:
                # e0[0:1,:] (ones row) x bo_pad[0:1,:] -> [128, D]
                bo_ps = pso.tile([128, D], F32, tag="out", name="bo_ps")
                nc.tensor.matmul(
                    bo_ps, lhsT=e0[0:1, :], rhs=bo_pad[0:1, :],
                    start=True, stop=True,
                )
                bo_bcast = consts.tile([128, D], F32, name="bo_bcast")
                nc.vector.tensor_copy(bo_bcast, bo_ps)

            # ---------------- K/V prep (once) ----------------
            # cT [128, 6, 128]; cols >= 77 zeroed directly (free-dim slice)
            cT_sb = prep.tile([128, 6, 128], BF16, name="cT_sb")
            nc.vector.memset(cT_sb[:, :, SKV:], 0.0)
            for g in range(2):
                cT_ps = pss.tile([128, 3, 128], BF16, tag="sT", name="cT_ps")
                for t3 in range(3):
                    t = g * 3 + t3
                    nc.tensor.transpose(
                        cT_ps[:, t3, :], ctx_sb[:, t * 128 : (t + 1) * 128], ident
                    )
                nc.vector.tensor_copy(
                    cT_sb[:, 3 * g : 3 * g + 3, :SKV], cT_ps[:, :, :SKV]
                )

            # kT per head, masked into pair halves:
            #   head h lives in partitions (h%2)*64..+64 of kT_sb[:, h, :];
            #   the other 64 partitions are zero; cols 77..127 zero.
            kT_sb = prep.tile([128, H, 128], BF16, name="kT_sb")
            nc.vector.memset(kT_sb, 0.0)
            for g in range(4):
                kT_ps = psq.tile([128, 128], F32, tag="q", name="kT_ps")
                for q in range(2):
                    h = 2 * g + q
                    off = q * 64
                    for ct in range(6):
                        nc.tensor.matmul(
                            kT_ps[off : off + 64, :],
                            lhsT=wk_sb[:, ct, h * 64 : (h + 1) * 64],
                            rhs=cT_sb[:, ct, :],
                            start=(ct == 0),
                            stop=(ct == 5),
                        )
                for q in range(2):
                    h = 2 * g + q
                    off = q * 64
                    nc.scalar.activation(
                        kT_sb[off : off + 64, h, :SKV],
                        kT_ps[off : off + 64, :SKV],
                        AF.Identity,
                        bias=bk_hms[off : off + 64, g : g + 1],
                        scale=0.125,
                    )

            # v [kv, d] with rows >= 77 exactly 0 (zero cT cols + e0[:, :77])
            # built lazily in body 0 (after front(0)) so qproj(0) does not
            # queue behind the wv DMA on the PE.
            state = {}

            def prep_v():
                v_ps = pso.tile([128, D], F32, tag="out", name="v_ps")
                for ct in range(6):
                    nc.tensor.matmul(
                        v_ps,
                        lhsT=cT_sb[:, ct, :],
                        rhs=wv_sb[:, ct, :],
                        start=(ct == 0),
                        stop=False,
                    )
                nc.tensor.matmul(
                    v_ps, lhsT=e0, rhs=bv_pad, start=False, stop=True
                )
                # rows >= 77 got bv from the rank-1 bias matmul; zero them
                # (they must mask the exp(0)=1 padding rows in PV).
                v_sb = prep.tile([128, D], BF16, name="v_sb")
                nc.vector.memset(v_sb[64:128, :], 0.0)
                nc.vector.tensor_copy(v_sb[:SKV, :], v_ps[:SKV, :])
                state["v_sb"] = v_sb

            # ---------------- main loop (software-pipelined) ----------
            # Emission order per body it:
            #   load x(it+2) | xpose(it+1) | qproj+scores+exp(it)
            #   | outproj(it-1)+store | sums/PV/normalize(it)
            # PE in-order queue: Q(it) S(it) O(it-1) Sums/PV(it) - each
            # cross-engine latency hop (qbias DVE, exp ACT, divide DVE)
            # is covered by the next block of independent PE work.
            xT_tiles = {}

            def xpose(it):
                if it >= n_iters:
                    return
                x_sb = x_tiles.pop(it)
                xT_sb = xpool.tile([128, 4, NROW], BF16, tag="xT", bufs=3,
                                   name="xT_sb")
                if XT_MODE == "xbar":
                    for c in range(NCH):
                        nc.sync.dma_start_transpose(
                            xT_sb[:, :, c * 128 : (c + 1) * 128], x_sb[:, c, :]
                        )
                else:
                    for c in range(NCH):
                        xT_ps = psq.tile([128, 4, 128], BF16, tag="q",
                                         name="xT_ps")
                        for et in range(4):
                            nc.tensor.transpose(
                                xT_ps[:, et, :],
                                x_sb[:, c, et * 128 : (et + 1) * 128],
                                ident,
                            )
                        nc.scalar.copy(xT_sb[:, :, c * 128 : (c + 1) * 128], xT_ps)
                xT_tiles[it] = xT_sb

            def stage_front(it):
                """qproj + scores + exp for iter it."""
                xT_sb = xT_tiles.pop(it)
                qT_sb = spool.tile([128, 4, NROW], BF16, tag="qT", name="qT_sb")
                for g in range(4):
                    qT_ps = psq.tile([128, NROW], F32, tag="q", name="qT_ps")
                    for et in range(4):
                        nc.tensor.matmul(
                            qT_ps,
                            lhsT=wq_sb[:, et, g * 128 : (g + 1) * 128],
                            rhs=xT_sb[:, et, :],
                            start=(et == 0),
                            stop=(et == 3),
                        )
                    # single-ALU-op add: a mult-by-1.0 + add costs ~100ns
                    # more per op and delays the DVE queue tail
                    nc.vector.tensor_scalar_add(
                        qT_sb[:, g, :], qT_ps, bq_pair[:, g : g + 1]
                    )

                expT_sb = spool.tile([128, H, NROW], BF16, tag="expT", name="expT_sb")
                for g in range(4):
                    sT_ps = pss.tile([128, 2, NROW], F32, tag="sT", name="sT_ps")
                    for q in range(2):
                        nc.tensor.matmul(
                            sT_ps[:, q, :],
                            lhsT=kT_sb[:, 2 * g + q, :],
                            rhs=qT_sb[:, g, :],
                            start=True,
                            stop=True,
                        )
                    nc.scalar.activation(
                        expT_sb[:, 2 * g : 2 * g + 2, :], sT_ps, AF.Exp
                    )
                return expT_sb

            def stage_back(it, expT_sb):
                """sums + PV + softmax normalize for iter it.

                Returns per-half attnT SBUF tiles [128, 2, NROW] (bf16):
                half h covers heads 4h..4h+3 packed (h%2)*64 partitions,
                free index dt-within-half.
                """
                halves = []
                for half in range(2):
                    sums_ps = pss.tile([128, 2, NROW], F32, tag="sT", name="sums_ps")
                    pv_ps = psw.tile([128, 2, NROW], F32, tag="pv", name="pv_ps")
                    for hh in range(4):
                        h = half * 4 + hh
                        dt, off = hh // 2, (h % 2) * 64
                        nc.tensor.matmul(
                            sums_ps[off : off + 64, dt, :],
                            lhsT=ones64,
                            rhs=expT_sb[:, h, :],
                            start=True,
                            stop=True,
                        )
                    for hh in range(4):
                        h = half * 4 + hh
                        dt, off = hh // 2, (h % 2) * 64
                        nc.tensor.matmul(
                            pv_ps[off : off + 64, dt, :],
                            lhsT=state["v_sb"][:, h * 64 : (h + 1) * 64],
                            rhs=expT_sb[:, h, :],
                            start=True,
                            stop=True,
                        )
                    rsum_sb = spool.tile([128, 2, NROW], F32, tag="rsum",
                                         name="rsum_sb")
                    nc.vector.reciprocal_approx_fast(rsum_sb, sums_ps)
                    a_sb = spool.tile([128, 2, NROW], BF16, tag="attnT", bufs=4,
                                      name="a_sb")
                    nc.vector.tensor_mul(a_sb, pv_ps, rsum_sb)
                    halves.append(a_sb)
                return halves

            def stage_out(it, halves):
                """out projection + bias + store for iter it."""
                for c in range(NCH):
                    out_ps = pso.tile([128, D], F32, tag="out", name="out_ps")
                    for half in range(2):
                        for j in range(2):
                            dt = 2 * half + j
                            nc.tensor.matmul(
                                out_ps,
                                lhsT=halves[half][:, j, c * 128 : (c + 1) * 128],
                                rhs=wo_sb[:, dt, :],
                                start=(dt == 0),
                                stop=(OB_MODE != "pe" and dt == 3),
                            )
                    out_sb = opool.tile([128, D], F32, tag="out", name="out_sb")
                    if OB_MODE == "dve":
                        # one DVE op: PSUM->SBUF copy + bo add. No ACT copy,
                        # no PE bias matmul - the scalar queue is exps-only.
                        nc.vector.tensor_tensor(out_sb, out_ps, bo_bcast, ALU.add)
                        st = out_sb
                    elif OB_MODE == "gp":
                        nc.scalar.copy(out_sb, out_ps)
                        out2_sb = opool.tile([128, D], F32, tag="out2",
                                             name="out2_sb")
                        nc.gpsimd.tensor_add(out2_sb, out_sb, bo_bcast)
                        st = out2_sb
                    else:
                        nc.tensor.matmul(
                            out_ps, lhsT=e0, rhs=bo_pad, start=False, stop=True
                        )
                        nc.scalar.copy(out_sb, out_ps)
                        st = out_sb
                    nc.sync.dma_start(
                        out_d[it * NROW + c * 128 : it * NROW + (c + 1) * 128, :],
                        st,
                    )

            xpose(0)
            xpose(1)
            prev = None
            for it in range(n_iters):
                load_x(it + 3)
                xpose(it + 2)
                expT = stage_front(it)
                if it == 0:
                    prep_v()
                if prev is not None:
                    stage_out(it - 1, prev)
                prev = stage_back(it, expT)
            stage_out(n_iters - 1, prev)

    nc.compile()
    return nc


_BUILD_CACHE = {}


def _get_nc():
    key = (XT_MODE, OB_MODE)
    if key not in _BUILD_CACHE:
        _BUILD_CACHE[key] = build_nc()
    return _BUILD_CACHE[key]


def _in_maps(latent, context, wq, bq, wk, bk, wv, bv, wo, bo):
    f = lambda a: np.ascontiguousarray(np.asarray(a), dtype=np.float32)
    shared = {
        "wq": f(wq), "bq": f(bq), "wk": f(wk), "bk": f(bk),
        "wv": f(wv), "bv": f(bv), "wo": f(wo), "bo": f(bo),
    }
    maps = []
    for b in range(N_CORES):
        m = dict(shared)
        m["latent"] = f(latent[b])
        m["context"] = f(context[b])
        maps.append(m)
    return maps


def run_on_hw(inputs, trace=False, **kw):
    nc = _get_nc()
    maps = _in_maps(**inputs)
    res = run_bass_kernel_spmd(nc, maps, list(range(N_CORES)), trace=trace, **kw)
    out = np.stack([res.results[b]["out"] for b in range(N_CORES)], axis=0)
    return out, res


def kernel(latent, context, wq, bq, wk, bk, wv, bv, wo, bo):
    out, _ = run_on_hw(dict(
        latent=latent, context=context, wq=wq, bq=bq, wk=wk, bk=bk,
        wv=wv, bv=bv, wo=wo, bo=bo,
    ))
    return out
